# revision 23
# baseline (speedup 1.0000x reference)
"""ConditioningMoEINR Trainium2 kernel — dense 8-core data-parallel.

Kernel (CoreSim ~748us vs 2082us for the all-fp32 predecessor):
- float32r matmuls for encoder + experts (1 cyc/row vs 4 for fp32).
  Operands must be *produced* as f32r: weights go through Pool-initiated
  casting DMAs, activations are written f32r by the Act engine. Policy +
  logits stay exact fp32 so top-2 routing never flips vs the reference.
- Software-pipelined emission: experts(g) interleaved 2:1 with trunk(g+1),
  experts advance in stage-interleaved pairs — per-engine queues execute
  in program order, so emission order is the schedule.
- Engine placement under the "GPSIMD cannot touch PSUM" rule: range wraps
  and magic-round ops (PSUM) on DVE, sins + PSUM-evacuating copies on Act,
  SBUF-only sp2->f32r casts on Pool.
- In-kernel transposing DMA of x (host ships the raw [N,4] array).

Dispatch: persistent jit + shard_map executable, device-resident weights
keyed by crc32 (uploaded once per weight-set), only x (2MB) moves per
call, and the previous output buffer is donated back as the next call's
output allocation, so steady-state host->device traffic is just x.
"""

import sys

if "/opt/trn_rl_repo" not in sys.path:
    sys.path.insert(0, "/opt/trn_rl_repo")

import zlib

import numpy as np

# ---- problem constants (hardcoded per contract) ----
N_TOTAL = 131072
N_CORES = 8
NPC = N_TOTAL // N_CORES          # 16384 points per core
CHUNK = 512                       # matmul moving-dim tile
NCHUNK = NPC // CHUNK             # 32
GROUP = 2                         # chunks per expert psum group
NGROUP = NCHUNK // GROUP          # 16
NSUB = CHUNK // 128               # 4 point-subtiles per chunk
NJ = NCHUNK * NSUB                # 128 point-major column groups
NUM_FREQ = 6
IN_F = 4
ENC = 128
POL = 64
EXP = 128
NE = 7
OMEGA = 30.0

PI = float(np.pi)
TWO_PI = float(2 * np.pi)
MAGIC = float(np.float32(1.5 * 2 ** 23))
S_PER = float(np.float32(OMEGA / (2 * np.pi)))   # radians->period units
BIG = 1.0e30

_RT = {}


def _build(npc):
    import concourse.bacc as bacc
    import concourse.mybir as mybir
    import concourse.tile as tile
    from contextlib import ExitStack

    DT = mybir.dt.float32
    F32R = mybir.dt.float32r
    AF = mybir.ActivationFunctionType
    ALU = mybir.AluOpType

    nchunk = npc // CHUNK
    ngroup = nchunk // GROUP
    nj = nchunk * NSUB

    nc = bacc.Bacc("TRN2", target_bir_lowering=False, debug=False)

    def din(name, shape):
        return nc.dram_tensor(name, list(shape), DT, kind="ExternalInput").ap()

    x_d = din("x", (npc, IN_F))               # natural row-major x shard
    pe_bs = din("pe_bs", (IN_F, 48))          # period-unit freq matrix
    pe_shift = din("pe_shift", (48, 1))       # 0 / 0.25 (cos rows)
    pe_bias = din("pe_bias", (48, 1))         # 0 / pi/2
    encW1a = din("encW1a", (48, ENC))         # sin/cos rows * 30/2pi
    encW1b = din("encW1b", (IN_F, ENC))       # x rows * 30/2pi
    encW2r = din("encW2r", (ENC, ENC))        # * 30
    polW0p = din("polW0p", (IN_F, POL))       # * 30/2pi
    polW1r = din("polW1r", (POL, POL))        # * 30
    polW2r = din("polW2r", (POL, POL))        # * 30
    polWl = din("polWl", (POL, NE))
    eW0a = din("eW0a", (ENC, NE * EXP))       # * 30, expert-major columns
    eW0b = din("eW0b", (POL, NE * EXP))       # * 30
    eW1 = din("eW1", (EXP, NE * EXP))         # * 30
    eW2 = din("eW2", (EXP, NE * EXP))         # * 30
    eWo = din("eWo", (EXP, NE * NE))          # col-e padded Wo
    ident = din("ident", (128, 128))
    out_d = nc.dram_tensor("out", [npc], DT, kind="ExternalOutput").ap()

    def r(ap):
        return ap.bitcast(F32R)

    with tile.TileContext(nc) as tc, ExitStack() as ctx:
        wpool = ctx.enter_context(tc.tile_pool(name="w", bufs=1))
        spool = ctx.enter_context(tc.tile_pool(name="s", bufs=3))
        s2pool = ctx.enter_context(tc.tile_pool(name="s2", bufs=4))
        gpool = ctx.enter_context(tc.tile_pool(name="g", bufs=3))
        rpool = ctx.enter_context(tc.tile_pool(name="r", bufs=1))
        ppool = ctx.enter_context(tc.tile_pool(name="ps", bufs=2, space="PSUM"))
        tpool = ctx.enter_context(tc.tile_pool(name="tr", bufs=1, space="PSUM"))

        # ---- resident weights ----
        _wn = [0]

        def wload(ap, shape, dt=DT):
            _wn[0] += 1
            t = wpool.tile(list(shape), dt, name=f"w{_wn[0]}", tag=f"w{_wn[0]}")
            # Pool-initiated DMAs can cast (fp32 -> f32r rounds on write)
            eng = nc.gpsimd if dt is F32R else nc.sync
            eng.dma_start(t[:], ap)
            return t

        w_pebs = wload(pe_bs[:], (IN_F, 48))
        w_peshift = wload(pe_shift[:], (48, 1))
        w_pebias = wload(pe_bias[:], (48, 1))
        w_enc1a = wload(encW1a[:], (48, ENC), F32R)
        w_enc1b = wload(encW1b[:], (IN_F, ENC))
        w_enc2 = wload(encW2r[:], (ENC, ENC), F32R)
        w_pol0 = wload(polW0p[:], (IN_F, POL))
        w_pol1 = wload(polW1r[:], (POL, POL))
        w_pol2 = wload(polW2r[:], (POL, POL))
        w_polWl = wload(polWl[:], (POL, NE))
        w_e0a_all = wload(eW0a[:], (ENC, NE * EXP), F32R)
        w_e0b_all = wload(eW0b[:], (POL, NE * EXP), F32R)
        w_e1_all = wload(eW1[:], (EXP, NE * EXP), F32R)
        w_e2_all = wload(eW2[:], (EXP, NE * EXP), F32R)
        w_eo_all = wload(eWo[:], (EXP, NE * NE), F32R)
        w_e0a = [w_e0a_all[:, e * EXP:(e + 1) * EXP] for e in range(NE)]
        w_e0b = [w_e0b_all[:, e * EXP:(e + 1) * EXP] for e in range(NE)]
        w_e1 = [w_e1_all[:, e * EXP:(e + 1) * EXP] for e in range(NE)]
        w_e2 = [w_e2_all[:, e * EXP:(e + 1) * EXP] for e in range(NE)]
        w_eo = [w_eo_all[:, e * NE:(e + 1) * NE] for e in range(NE)]
        w_id = wload(ident[:], (128, 128))

        xT_dram = x_d.rearrange("n f -> f n")

        # point-major logits / preds for the whole core
        LT = rpool.tile([128, nj * NE], DT)
        PT = rpool.tile([128, nj * NE], DT)

        def magic_sin(dst, psum_ap, p, n, shift_ap, bias_ap, eng, pool=None):
            """dst = Sin(-2pi*((psum+shift+M)-M-psum) + bias); psum in period units."""
            pool = pool or spool
            tsh = pool.tile([p, n], DT, tag="tsh")
            if shift_ap is None:
                eng.tensor_scalar_add(tsh[:p, :n], psum_ap, MAGIC)
            else:
                eng.tensor_scalar(
                    tsh[:p, :n], psum_ap, shift_ap, MAGIC, op0=ALU.add, op1=ALU.add
                )
            u = pool.tile([p, n], DT, tag="u")
            eng.scalar_tensor_tensor(
                u[:p, :n], tsh[:p, :n], MAGIC, psum_ap,
                op0=ALU.subtract, op1=ALU.subtract,
            )
            if bias_ap is None:
                nc.scalar.activation(dst, u[:p, :n], AF.Sin, scale=-TWO_PI)
            else:
                nc.scalar.activation(dst, u[:p, :n], AF.Sin, bias=bias_ap, scale=-TWO_PI)

        def wrap_sin(dst, psum_ap, p, n):
            """dst = Sin(wrap(psum)); psum in radians, |arg| < 3pi."""
            nc.vector.add_range_wrap(psum_ap, psum_ap, shift=0.0, bound=PI, period=TWO_PI)
            nc.scalar.activation(dst, psum_ap, AF.Sin)

        state = {}

        def trunk_gen(g):
            """Trunk (pe/encoder/policy/logits) for group g; yields between stages."""
            xg = spool.tile([IN_F, GROUP * CHUNK], DT, tag="xg")
            nc.sync.dma_start(
                xg[:], xT_dram[:, g * GROUP * CHUNK:(g + 1) * GROUP * CHUNK]
            )
            s2s = []
            sp2s = []
            state[g] = (s2s, sp2s)
            for ci in range(GROUP):
                c = g * GROUP + ci
                xs = xg[:, ci * CHUNK:(ci + 1) * CHUNK]

                # --- positional encoding (f32r) ---
                t48 = ppool.tile([128, CHUNK], DT, tag="trunk")
                nc.tensor.matmul(t48[0:48, :], w_pebs[:], xs, start=True, stop=True)
                pesin = spool.tile([48, CHUNK], F32R, tag="pesin")
                magic_sin(pesin[:], t48[0:48, :], 48, CHUNK, w_peshift[:], w_pebias[:],
                          nc.vector)
                yield

                # --- encoder (f32r) ---
                h1 = ppool.tile([128, CHUNK], DT, tag="trunk")
                nc.tensor.matmul(h1[:], w_enc1a[:], pesin[:], start=True, stop=False)
                nc.tensor.matmul(h1[:], w_enc1b[:], xs, start=False, stop=True)
                s1 = spool.tile([ENC, CHUNK], F32R, tag="s1")
                magic_sin(s1[:], h1[:], ENC, CHUNK, None, None, nc.vector)
                yield

                h2 = ppool.tile([128, CHUNK], DT, tag="trunk")
                nc.tensor.matmul(h2[:], w_enc2[:], s1[:], start=True, stop=True)
                s2 = s2pool.tile([ENC, CHUNK], F32R, tag="s2")
                wrap_sin(s2[:], h2[:], ENC, CHUNK)
                s2s.append(s2)
                yield

                # --- policy (exact fp32; feeds routing) ---
                p0 = ppool.tile([128, CHUNK], DT, tag="trunk")
                nc.tensor.matmul(p0[0:POL, :], w_pol0[:], xs, start=True, stop=True)
                sp0 = spool.tile([POL, CHUNK], DT, tag="sp0")
                magic_sin(sp0[:], p0[0:POL, :], POL, CHUNK, None, None, nc.vector)
                yield

                p1 = ppool.tile([128, CHUNK], DT, tag="trunk")
                nc.tensor.matmul(p1[0:POL, :], w_pol1[:], sp0[:], start=True, stop=True)
                sp1 = spool.tile([POL, CHUNK], DT, tag="sp1")
                wrap_sin(sp1[:], p1[0:POL, :], POL, CHUNK)
                yield

                p2 = ppool.tile([128, CHUNK], DT, tag="trunk")
                nc.tensor.matmul(p2[0:POL, :], w_pol2[:], sp1[:], start=True, stop=True)
                sp2 = s2pool.tile([POL, CHUNK], DT, tag="sp2")
                wrap_sin(sp2[:], p2[0:POL, :], POL, CHUNK)
                sp2r = s2pool.tile([POL, CHUNK], F32R, tag="sp2r")
                nc.gpsimd.tensor_copy(sp2r[:], sp2[:])
                sp2s.append(sp2r)
                yield

                # --- logits, point-major [128, 28] (fp32) ---
                lt = ppool.tile([128, NSUB * NE], DT, tag="trunk")
                for s in range(NSUB):
                    nc.tensor.matmul(
                        lt[:, s * NE:(s + 1) * NE],
                        sp2[:, s * 128:(s + 1) * 128],
                        w_polWl[:],
                        start=True, stop=True,
                    )
                nc.scalar.activation(LT[:, c * NSUB * NE:(c + 1) * NSUB * NE], lt[:, 0:NSUB * NE], AF.Copy)
                yield

        def experts_gen(g):
            """Expert block for group g (f32r); experts advance in stage-
            interleaved pairs so every engine always has two independent ops
            queued. Yields between stage waves."""
            s2s, sp2s = state.pop(g)
            p7 = tpool.tile([NE, GROUP * CHUNK], DT, tag="p7")
            for pair in ((0, 1), (2, 3), (4, 5), (6,)):
                x0s, g0s, x1s, g1s, x2s, g2s = {}, {}, {}, {}, {}, {}
                for e in pair:
                    x0 = x0s[e] = ppool.tile([128, GROUP * CHUNK], DT, tag="exp", name=f"x0_{g}_{e}")
                    for ci in range(GROUP):
                        sl = x0[:, ci * CHUNK:(ci + 1) * CHUNK]
                        nc.tensor.matmul(sl, w_e0a[e], s2s[ci][:], start=True, stop=False)
                        nc.tensor.matmul(sl, w_e0b[e], sp2s[ci][:], start=False, stop=True)
                yield
                for e in pair:
                    g0 = g0s[e] = gpool.tile([EXP, GROUP * CHUNK], F32R, tag="g0", name=f"g0_{g}_{e}")
                    wrap_sin(g0[:], x0s[e][:], EXP, GROUP * CHUNK)
                yield
                for e in pair:
                    x1 = x1s[e] = ppool.tile([128, GROUP * CHUNK], DT, tag="exp", name=f"x1_{g}_{e}")
                    for ci in range(GROUP):
                        nc.tensor.matmul(
                            x1[:, ci * CHUNK:(ci + 1) * CHUNK], w_e1[e],
                            g0s[e][:, ci * CHUNK:(ci + 1) * CHUNK], start=True, stop=True,
                        )
                yield
                for e in pair:
                    g1 = g1s[e] = gpool.tile([EXP, GROUP * CHUNK], F32R, tag="g1", name=f"g1_{g}_{e}")
                    wrap_sin(g1[:], x1s[e][:], EXP, GROUP * CHUNK)
                yield
                for e in pair:
                    x2 = x2s[e] = ppool.tile([128, GROUP * CHUNK], DT, tag="exp", name=f"x2_{g}_{e}")
                    for ci in range(GROUP):
                        nc.tensor.matmul(
                            x2[:, ci * CHUNK:(ci + 1) * CHUNK], w_e2[e],
                            g1s[e][:, ci * CHUNK:(ci + 1) * CHUNK], start=True, stop=True,
                        )
                yield
                for e in pair:
                    g2 = g2s[e] = gpool.tile([EXP, GROUP * CHUNK], F32R, tag="g2", name=f"g2_{g}_{e}")
                    wrap_sin(g2[:], x2s[e][:], EXP, GROUP * CHUNK)
                yield
                for e in pair:
                    for ci in range(GROUP):
                        nc.tensor.matmul(
                            p7[0:NE, ci * CHUNK:(ci + 1) * CHUNK], w_eo[e],
                            g2s[e][:, ci * CHUNK:(ci + 1) * CHUNK],
                            start=(e == 0), stop=(e == NE - 1),
                        )
                yield

            # preds -> point-major PT via PE transpose
            for ci in range(GROUP):
                c = g * GROUP + ci
                tmp7 = spool.tile([NE, CHUNK], DT, tag="tmp7")
                nc.scalar.activation(tmp7[:], p7[0:NE, ci * CHUNK:(ci + 1) * CHUNK], AF.Copy)
                tp = ppool.tile([128, NSUB * NE], DT, tag="trunk")
                for s in range(NSUB):
                    nc.tensor.transpose(
                        tp[:, s * NE:(s + 1) * NE],
                        tmp7[:, s * 128:(s + 1) * 128],
                        w_id[0:NE, 0:NE],
                    )
                nc.scalar.activation(PT[:, c * NSUB * NE:(c + 1) * NSUB * NE], tp[:, 0:NSUB * NE], AF.Copy)
                yield

        _STOP = object()

        # ===== routing + combine, column-sliced so the first half can be
        # emitted mid-pipeline (hidden under groups ngroup/2.. expert work).
        def routing_block(jl, jh, sfx):
            w = jh - jl
            LTs = LT[:, jl * NE:jh * NE]
            PTs = PT[:, jl * NE:jh * NE]
            LT3 = LTs.rearrange("p (j e) -> p j e", e=NE)

            def etree(op, src3, width_tag):
                m4 = rpool.tile([128, w * 4], DT, tag=f"{width_tag}4{sfx}")
                m43 = m4[:].rearrange("p (j e) -> p j e", e=4)
                nc.vector.tensor_tensor(m43, src3[:, :, 0:4], src3[:, :, 3:7], op)
                m2 = rpool.tile([128, w * 2], DT, tag=f"{width_tag}2{sfx}")
                m23 = m2[:].rearrange("p (j e) -> p j e", e=2)
                nc.vector.tensor_tensor(m23, m43[:, :, 0:2], m43[:, :, 2:4], op)
                m1 = rpool.tile([128, w], DT, tag=f"{width_tag}1{sfx}")
                m13 = m1[:].rearrange("p (j e) -> p j e", e=1)
                nc.vector.tensor_tensor(m13, m23[:, :, 0:1], m23[:, :, 1:2], op)
                return m1

            def erep(m1, tag):
                r_ = rpool.tile([128, w * NE], DT, tag=f"{tag}{sfx}")
                r3 = r_[:].rearrange("p (j e) -> p j e", e=NE)
                m13 = m1[:].rearrange("p (j e) -> p j e", e=1)
                nc.gpsimd.tensor_copy(r3[:, :, 0:1], m13)
                nc.gpsimd.tensor_copy(r3[:, :, 1:2], r3[:, :, 0:1])
                nc.gpsimd.tensor_copy(r3[:, :, 2:4], r3[:, :, 0:2])
                nc.gpsimd.tensor_copy(r3[:, :, 4:7], r3[:, :, 1:4])
                return r_, r3

            mx1 = etree(ALU.max, LT3, "mxa")
            rep1, _ = erep(mx1, "rep1")
            ge1 = rpool.tile([128, w * NE], DT, tag=f"ge1{sfx}")
            nc.vector.tensor_tensor(ge1[:], LTs, rep1[:], ALU.is_ge)
            maskd = rpool.tile([128, w * NE], DT, tag=f"maskd{sfx}")
            nc.vector.scalar_tensor_tensor(
                maskd[:], ge1[:], BIG, LTs, op0=ALU.mult, op1=ALU.subtract
            )
            mn = etree(ALU.min, maskd[:].rearrange("p (j e) -> p j e", e=NE), "mna")
            mx2 = rpool.tile([128, w], DT, tag=f"mx2{sfx}")
            nc.vector.tensor_scalar_mul(mx2[:], mn[:], -1.0)
            rep2, _ = erep(mx2, "rep2")
            keep = rpool.tile([128, w * NE], DT, tag=f"keep{sfx}")
            nc.vector.tensor_tensor(keep[:], LTs, rep2[:], ALU.is_ge)

            ex = rpool.tile([128, w * NE], DT, tag=f"ex{sfx}")
            nc.scalar.activation(ex[:], LTs, AF.Exp)
            ew = rpool.tile([128, w * NE], DT, tag=f"ew{sfx}")
            nc.vector.tensor_tensor(ew[:], ex[:], keep[:], ALU.mult)
            wp = rpool.tile([128, w * NE], DT, tag=f"wp{sfx}")
            nc.vector.tensor_tensor(wp[:], ew[:], PTs, ALU.mult)

            den = rpool.tile([128, w], DT, tag=f"den{sfx}")
            nc.vector.tensor_reduce(
                den[:], ew[:].rearrange("p (j e) -> p j e", e=NE),
                mybir.AxisListType.X, ALU.add,
            )
            num = rpool.tile([128, w], DT, tag=f"num{sfx}")
            nc.vector.tensor_reduce(
                num[:], wp[:].rearrange("p (j e) -> p j e", e=NE),
                mybir.AxisListType.X, ALU.add,
            )
            rec = rpool.tile([128, w], DT, tag=f"rec{sfx}")
            scratch = rpool.tile([128, w], DT, tag=f"recs{sfx}")
            nc.vector.reciprocal_approx_accurate(rec[:], den[:], scratch[:])
            outv = rpool.tile([128, w], DT, tag=f"outv{sfx}")
            nc.vector.tensor_tensor(outv[:], num[:], rec[:], ALU.mult)

            # transpose [128 q, w] -> [w, 128 q] and store
            tp = ppool.tile([128, 128], DT, tag="exp")
            nc.tensor.transpose(tp[0:w, 0:128], outv[:, 0:w], w_id[:])
            osb = rpool.tile([w, 128], DT, tag=f"osb{sfx}")
            nc.scalar.activation(osb[:], tp[0:w, 0:128], AF.Copy)
            nc.sync.dma_start(
                out_d.rearrange("(j q) -> j q", q=128)[jl:jh, :], osb[:],
            )

        # software pipeline: experts(g) interleaved with trunk(g+1); the
        # per-engine queues execute in program order, so the emission order
        # must alternate independent work to keep every engine fed.
        # expert stream: ~30 yields/group; trunk: ~14 -> interleave 2:1.
        tg = trunk_gen(0)
        while next(tg, _STOP) is not _STOP:
            pass
        for g in range(ngroup):
            eg = experts_gen(g)
            tg = trunk_gen(g + 1) if g + 1 < ngroup else None
            alive = True
            while alive:
                e_live = next(eg, _STOP) is not _STOP
                e_live = (next(eg, _STOP) is not _STOP) or e_live
                t_live = tg is not None and next(tg, _STOP) is not _STOP
                alive = e_live or t_live
            if g == ngroup // 2 - 1:
                # columns 0..nj/2 are final; overlap their routing with the
                # remaining groups' expert compute.
                routing_block(0, nj // 2, "a")
        routing_block(nj // 2, nj, "b")

    nc.compile()
    return nc


_W_NAMES = [
    "enc_W1", "enc_W2", "pol_W0", "pol_W1", "pol_W2", "pol_Wl",
    "exp_W0", "exp_W1", "exp_W2", "exp_Wo",
]


def _prep_weights(inputs):
    """Per-core weight tensors (identical on every core)."""
    f32 = np.float32
    S30 = f32(OMEGA)
    SP = f32(OMEGA / (2 * np.pi))

    # pe freq matrix in period units: col j=i*6+k (sin), 24+j (cos) = 2^(k-1)
    pe_bs = np.zeros((IN_F, 48), f32)
    for i in range(IN_F):
        for k in range(NUM_FREQ):
            pe_bs[i, i * NUM_FREQ + k] = 2.0 ** (k - 1)
            pe_bs[i, 24 + i * NUM_FREQ + k] = 2.0 ** (k - 1)
    pe_shift = np.zeros((48, 1), f32)
    pe_shift[24:48] = 0.25
    pe_bias = (pe_shift * f32(2 * np.pi)).astype(f32)

    # enc_W1 rows permuted to [sin/cos(48); x(4)], scaled to period units
    encW1 = np.asarray(inputs["enc_W1"], f32)
    encW1p = np.concatenate([encW1[4:52], encW1[0:4]], axis=0) * SP

    d = {
        "pe_bs": pe_bs,
        "pe_shift": pe_shift,
        "pe_bias": pe_bias,
        "encW1a": np.ascontiguousarray(encW1p[0:48]).astype(f32),
        "encW1b": np.ascontiguousarray(encW1p[48:52]).astype(f32),
        "encW2r": (np.asarray(inputs["enc_W2"], f32) * S30),
        "polW0p": (np.asarray(inputs["pol_W0"], f32)[0:IN_F] * SP),
        "polW1r": (np.asarray(inputs["pol_W1"], f32) * S30),
        "polW2r": (np.asarray(inputs["pol_W2"], f32) * S30),
        "polWl": np.asarray(inputs["pol_Wl"], f32),
        "eW0a": np.ascontiguousarray(
            (np.asarray(inputs["exp_W0"], f32)[:, 0:ENC, :] * S30)
            .transpose(1, 0, 2).reshape(ENC, NE * EXP)),
        "eW0b": np.ascontiguousarray(
            (np.asarray(inputs["exp_W0"], f32)[:, ENC:ENC + POL, :] * S30)
            .transpose(1, 0, 2).reshape(POL, NE * EXP)),
        "eW1": np.ascontiguousarray(
            (np.asarray(inputs["exp_W1"], f32) * S30)
            .transpose(1, 0, 2).reshape(EXP, NE * EXP)),
        "eW2": np.ascontiguousarray(
            (np.asarray(inputs["exp_W2"], f32) * S30)
            .transpose(1, 0, 2).reshape(EXP, NE * EXP)),
        "ident": np.eye(128, dtype=f32),
    }
    eWo = np.zeros((EXP, NE, NE), f32)
    for e in range(NE):
        eWo[:, e, e] = np.asarray(inputs["exp_Wo"], f32)[e, :, 0]
    d["eWo"] = eWo.reshape(EXP, NE * NE)

    # biases are structurally zero in this model; the kernel folds none.
    for b in ["enc_b1", "enc_b2", "pol_b0", "pol_b1", "pol_b2", "pol_bl",
              "exp_b0", "exp_b1", "exp_b2", "exp_bo"]:
        assert not np.any(np.asarray(inputs[b])), f"nonzero bias {b} unsupported"

    return d


def _runtime():
    """Build (once) the bass module and a persistent jitted 8-core executable."""
    if "sharded" in _RT:
        return _RT

    import jax
    import concourse.mybir as mybir
    from concourse.bass2jax import _bass_exec_p, install_neuronx_cc_hook

    import warnings

    with warnings.catch_warnings():
        warnings.simplefilter("ignore")
        from jax.experimental.shard_map import shard_map
    from jax.sharding import Mesh, NamedSharding, PartitionSpec

    install_neuronx_cc_hook()

    nc = _build(NPC)

    partition_name = nc.partition_id_tensor.name if nc.partition_id_tensor else None
    in_names, out_names, out_avals = [], [], []
    for alloc in nc.m.functions[0].allocations:
        if not isinstance(alloc, mybir.MemoryLocationSet):
            continue
        name = alloc.memorylocations[0].name
        if alloc.kind == "ExternalInput":
            if name != partition_name:
                in_names.append(name)
        elif alloc.kind == "ExternalOutput":
            out_names.append(name)
            out_avals.append(
                jax.core.ShapedArray(tuple(alloc.tensor_shape), mybir.dt.np(alloc.dtype))
            )
    n_params = len(in_names)
    n_outs = len(out_names)
    in_names_all = list(in_names) + list(out_names)
    if partition_name is not None:
        in_names_all.append(partition_name)
    donate = tuple(range(n_params, n_params + n_outs))

    def _body(*args):
        operands = list(args)
        if partition_name is not None:
            from concourse.bass2jax import partition_id_tensor

            operands.append(partition_id_tensor())
        outs = _bass_exec_p.bind(
            *operands,
            out_avals=tuple(out_avals),
            in_names=tuple(in_names_all),
            out_names=tuple(out_names),
            lowering_input_output_aliases=(),
            sim_require_finite=True,
            sim_require_nnan=True,
            nc=nc,
        )
        return tuple(outs)

    devices = jax.devices()[:N_CORES]
    mesh = Mesh(np.asarray(devices), ("core",))
    spec = PartitionSpec("core")
    sharded = jax.jit(
        shard_map(
            _body,
            mesh=mesh,
            in_specs=(spec,) * (n_params + n_outs),
            out_specs=(spec,) * n_outs,
            check_rep=False,
        ),
        donate_argnums=donate,
        keep_unused=True,
    )

    _RT.update(
        nc=nc, jax=jax, mesh=mesh, shard=NamedSharding(mesh, spec),
        sharded=sharded, in_names=in_names, out_avals=out_avals,
    )
    return _RT


def _fingerprint(inputs):
    acc = 0
    for k in _W_NAMES:
        a = np.ascontiguousarray(np.asarray(inputs[k], np.float32))
        acc = zlib.crc32(a.view(np.uint8).reshape(-1), acc)
    return acc


def kernel(**inputs):
    rt = _runtime()
    jax = rt["jax"]

    wfp = _fingerprint(inputs)
    if rt.get("wfp") != wfp:
        wd = _prep_weights(inputs)
        devw = {}
        for name in rt["in_names"]:
            if name == "x":
                continue
            a = wd[name]
            glob = np.concatenate([a] * N_CORES, axis=0)
            devw[name] = jax.device_put(glob, rt["shard"])
        rt["devw"] = devw
        rt["wfp"] = wfp
        rt["prev_out"] = None

    x = np.ascontiguousarray(np.asarray(inputs["x"], np.float32))
    assert x.shape == (N_TOTAL, IN_F)
    x_dev = jax.device_put(x, rt["shard"])

    donate_buf = rt.get("prev_out")
    if donate_buf is None:
        donate_buf = jax.device_put(np.zeros(N_TOTAL, np.float32), rt["shard"])

    args = [x_dev if n == "x" else rt["devw"][n] for n in rt["in_names"]]
    rt["prev_out"] = None  # donate_buf is consumed even if the call fails
    outs = rt["sharded"](*args, donate_buf)
    rt["prev_out"] = outs[0]
    res = np.asarray(outs[0])
    return res.reshape(N_TOTAL, 1).astype(np.float32, copy=False)


# revision 24
# speedup vs baseline: 1.2176x; 1.2176x over previous
"""ConditioningMoEINR Trainium2 kernel — dense 8-core data-parallel.

Kernel (CoreSim ~748us vs 2082us for the all-fp32 predecessor):
- float32r matmuls for encoder + experts (1 cyc/row vs 4 for fp32).
  Operands must be *produced* as f32r: weights go through Pool-initiated
  casting DMAs, activations are written f32r by the Act engine. Policy +
  logits stay exact fp32 so top-2 routing never flips vs the reference.
- Software-pipelined emission: experts(g) interleaved 2:1 with trunk(g+1),
  experts advance in stage-interleaved pairs — per-engine queues execute
  in program order, so emission order is the schedule.
- Engine placement under the "GPSIMD cannot touch PSUM" rule: range wraps
  and magic-round ops (PSUM) on DVE, sins + PSUM-evacuating copies on Act,
  SBUF-only sp2->f32r casts on Pool.
- In-kernel transposing DMA of x (host ships the raw [N,4] array).

Dispatch: persistent jit + shard_map executable, device-resident weights
keyed by crc32 (uploaded once per weight-set), only x (2MB) moves per
call, and the previous output buffer is donated back as the next call's
output allocation, so steady-state host->device traffic is just x.
"""

import sys

if "/opt/trn_rl_repo" not in sys.path:
    sys.path.insert(0, "/opt/trn_rl_repo")

import zlib

import numpy as np

# ---- problem constants (hardcoded per contract) ----
N_TOTAL = 131072
N_CORES = 8
NPC = N_TOTAL // N_CORES          # 16384 points per core
CHUNK = 512                       # matmul moving-dim tile
NCHUNK = NPC // CHUNK             # 32
GROUP = 2                         # chunks per expert psum group
NGROUP = NCHUNK // GROUP          # 16
NSUB = CHUNK // 128               # 4 point-subtiles per chunk
NJ = NCHUNK * NSUB                # 128 point-major column groups
NUM_FREQ = 6
IN_F = 4
ENC = 128
POL = 64
EXP = 128
NE = 7
OMEGA = 30.0

PI = float(np.pi)
TWO_PI = float(2 * np.pi)
MAGIC = float(np.float32(1.5 * 2 ** 23))
S_PER = float(np.float32(OMEGA / (2 * np.pi)))   # radians->period units
BIG = 1.0e30

_RT = {}


def _build(npc):
    import concourse.bacc as bacc
    import concourse.mybir as mybir
    import concourse.tile as tile
    from contextlib import ExitStack

    DT = mybir.dt.float32
    F32R = mybir.dt.float32r
    AF = mybir.ActivationFunctionType
    ALU = mybir.AluOpType

    nchunk = npc // CHUNK
    ngroup = nchunk // GROUP
    nj = nchunk * NSUB

    nc = bacc.Bacc("TRN2", target_bir_lowering=False, debug=False)

    def din(name, shape):
        return nc.dram_tensor(name, list(shape), DT, kind="ExternalInput").ap()

    x_d = din("x", (npc, IN_F))               # natural row-major x shard
    pe_bs = din("pe_bs", (IN_F, 48))          # period-unit freq matrix
    pe_shift = din("pe_shift", (48, 1))       # 0 / 0.25 (cos rows)
    pe_bias = din("pe_bias", (48, 1))         # 0 / pi/2
    encW1a = din("encW1a", (48, ENC))         # sin/cos rows * 30/2pi
    encW1b = din("encW1b", (IN_F, ENC))       # x rows * 30/2pi
    encW2r = din("encW2r", (ENC, ENC))        # * 30
    polW0p = din("polW0p", (IN_F, POL))       # * 30/2pi
    polW1r = din("polW1r", (POL, POL))        # * 30
    polW2r = din("polW2r", (POL, POL))        # * 30
    polWl = din("polWl", (POL, NE))
    eW0a = din("eW0a", (ENC, NE * EXP))       # * 30, expert-major columns
    eW0b = din("eW0b", (POL, NE * EXP))       # * 30
    eW1 = din("eW1", (EXP, NE * EXP))         # * 30
    eW2 = din("eW2", (EXP, NE * EXP))         # * 30
    eWo = din("eWo", (EXP, NE * NE))          # col-e padded Wo
    ident = din("ident", (128, 128))
    out_d = nc.dram_tensor("out", [npc], DT, kind="ExternalOutput").ap()

    def r(ap):
        return ap.bitcast(F32R)

    with tile.TileContext(nc) as tc, ExitStack() as ctx:
        wpool = ctx.enter_context(tc.tile_pool(name="w", bufs=1))
        spool = ctx.enter_context(tc.tile_pool(name="s", bufs=3))
        s2pool = ctx.enter_context(tc.tile_pool(name="s2", bufs=4))
        gpool = ctx.enter_context(tc.tile_pool(name="g", bufs=3))
        rpool = ctx.enter_context(tc.tile_pool(name="r", bufs=1))
        ppool = ctx.enter_context(tc.tile_pool(name="ps", bufs=2, space="PSUM"))
        tpool = ctx.enter_context(tc.tile_pool(name="tr", bufs=1, space="PSUM"))

        # ---- resident weights ----
        _wn = [0]

        def wload(ap, shape, dt=DT):
            _wn[0] += 1
            t = wpool.tile(list(shape), dt, name=f"w{_wn[0]}", tag=f"w{_wn[0]}")
            # Pool-initiated DMAs can cast (fp32 -> f32r rounds on write)
            eng = nc.gpsimd if dt is F32R else nc.sync
            eng.dma_start(t[:], ap)
            return t

        w_pebs = wload(pe_bs[:], (IN_F, 48))
        w_peshift = wload(pe_shift[:], (48, 1))
        w_pebias = wload(pe_bias[:], (48, 1))
        w_enc1a = wload(encW1a[:], (48, ENC), F32R)
        w_enc1b = wload(encW1b[:], (IN_F, ENC))
        w_enc2 = wload(encW2r[:], (ENC, ENC), F32R)
        w_pol0 = wload(polW0p[:], (IN_F, POL))
        w_pol1 = wload(polW1r[:], (POL, POL))
        w_pol2 = wload(polW2r[:], (POL, POL))
        w_polWl = wload(polWl[:], (POL, NE))
        w_e0a_all = wload(eW0a[:], (ENC, NE * EXP), F32R)
        w_e0b_all = wload(eW0b[:], (POL, NE * EXP), F32R)
        w_e1_all = wload(eW1[:], (EXP, NE * EXP), F32R)
        w_e2_all = wload(eW2[:], (EXP, NE * EXP), F32R)
        w_eo_all = wload(eWo[:], (EXP, NE * NE), F32R)
        w_e0a = [w_e0a_all[:, e * EXP:(e + 1) * EXP] for e in range(NE)]
        w_e0b = [w_e0b_all[:, e * EXP:(e + 1) * EXP] for e in range(NE)]
        w_e1 = [w_e1_all[:, e * EXP:(e + 1) * EXP] for e in range(NE)]
        w_e2 = [w_e2_all[:, e * EXP:(e + 1) * EXP] for e in range(NE)]
        w_eo = [w_eo_all[:, e * NE:(e + 1) * NE] for e in range(NE)]
        w_id = wload(ident[:], (128, 128))

        xT_dram = x_d.rearrange("n f -> f n")

        # point-major logits / preds for the whole core
        LT = rpool.tile([128, nj * NE], DT)
        PT = rpool.tile([128, nj * NE], DT)

        def magic_sin(dst, psum_ap, p, n, shift_ap, bias_ap, eng, pool=None):
            """dst = Sin(-2pi*((psum+shift+M)-M-psum) + bias); psum in period units."""
            pool = pool or spool
            tsh = pool.tile([p, n], DT, tag="tsh")
            if shift_ap is None:
                eng.tensor_scalar_add(tsh[:p, :n], psum_ap, MAGIC)
            else:
                eng.tensor_scalar(
                    tsh[:p, :n], psum_ap, shift_ap, MAGIC, op0=ALU.add, op1=ALU.add
                )
            u = pool.tile([p, n], DT, tag="u")
            eng.scalar_tensor_tensor(
                u[:p, :n], tsh[:p, :n], MAGIC, psum_ap,
                op0=ALU.subtract, op1=ALU.subtract,
            )
            if bias_ap is None:
                nc.scalar.activation(dst, u[:p, :n], AF.Sin, scale=-TWO_PI)
            else:
                nc.scalar.activation(dst, u[:p, :n], AF.Sin, bias=bias_ap, scale=-TWO_PI)

        def wrap_sin(dst, psum_ap, p, n):
            """dst = Sin(wrap(psum)); psum in radians, |arg| < 3pi."""
            nc.vector.add_range_wrap(psum_ap, psum_ap, shift=0.0, bound=PI, period=TWO_PI)
            nc.scalar.activation(dst, psum_ap, AF.Sin)

        state = {}

        def trunk_gen(g):
            """Trunk (pe/encoder/policy/logits) for group g; yields between stages."""
            xg = spool.tile([IN_F, GROUP * CHUNK], DT, tag="xg")
            nc.sync.dma_start(
                xg[:], xT_dram[:, g * GROUP * CHUNK:(g + 1) * GROUP * CHUNK]
            )
            s2s = []
            sp2s = []
            state[g] = (s2s, sp2s)
            for ci in range(GROUP):
                c = g * GROUP + ci
                xs = xg[:, ci * CHUNK:(ci + 1) * CHUNK]

                # --- positional encoding (f32r) ---
                t48 = ppool.tile([128, CHUNK], DT, tag="trunk")
                nc.tensor.matmul(t48[0:48, :], w_pebs[:], xs, start=True, stop=True)
                pesin = spool.tile([48, CHUNK], F32R, tag="pesin")
                magic_sin(pesin[:], t48[0:48, :], 48, CHUNK, w_peshift[:], w_pebias[:],
                          nc.vector)
                yield

                # --- encoder (f32r) ---
                h1 = ppool.tile([128, CHUNK], DT, tag="trunk")
                nc.tensor.matmul(h1[:], w_enc1a[:], pesin[:], start=True, stop=False)
                nc.tensor.matmul(h1[:], w_enc1b[:], xs, start=False, stop=True)
                s1 = spool.tile([ENC, CHUNK], F32R, tag="s1")
                magic_sin(s1[:], h1[:], ENC, CHUNK, None, None, nc.vector)
                yield

                h2 = ppool.tile([128, CHUNK], DT, tag="trunk")
                nc.tensor.matmul(h2[:], w_enc2[:], s1[:], start=True, stop=True)
                s2 = s2pool.tile([ENC, CHUNK], F32R, tag="s2")
                wrap_sin(s2[:], h2[:], ENC, CHUNK)
                s2s.append(s2)
                yield

                # --- policy (exact fp32; feeds routing) ---
                p0 = ppool.tile([128, CHUNK], DT, tag="trunk")
                nc.tensor.matmul(p0[0:POL, :], w_pol0[:], xs, start=True, stop=True)
                sp0 = spool.tile([POL, CHUNK], DT, tag="sp0")
                magic_sin(sp0[:], p0[0:POL, :], POL, CHUNK, None, None, nc.vector)
                yield

                p1 = ppool.tile([128, CHUNK], DT, tag="trunk")
                nc.tensor.matmul(p1[0:POL, :], w_pol1[:], sp0[:], start=True, stop=True)
                sp1 = spool.tile([POL, CHUNK], DT, tag="sp1")
                wrap_sin(sp1[:], p1[0:POL, :], POL, CHUNK)
                yield

                p2 = ppool.tile([128, CHUNK], DT, tag="trunk")
                nc.tensor.matmul(p2[0:POL, :], w_pol2[:], sp1[:], start=True, stop=True)
                sp2 = s2pool.tile([POL, CHUNK], DT, tag="sp2")
                wrap_sin(sp2[:], p2[0:POL, :], POL, CHUNK)
                sp2r = s2pool.tile([POL, CHUNK], F32R, tag="sp2r")
                nc.gpsimd.tensor_copy(sp2r[:], sp2[:])
                sp2s.append(sp2r)
                yield

                # --- logits, point-major [128, 28] (fp32) ---
                lt = ppool.tile([128, NSUB * NE], DT, tag="trunk")
                for s in range(NSUB):
                    nc.tensor.matmul(
                        lt[:, s * NE:(s + 1) * NE],
                        sp2[:, s * 128:(s + 1) * 128],
                        w_polWl[:],
                        start=True, stop=True,
                    )
                nc.scalar.activation(LT[:, c * NSUB * NE:(c + 1) * NSUB * NE], lt[:, 0:NSUB * NE], AF.Copy)
                yield

        def experts_gen(g):
            """Expert block for group g (f32r); experts advance in stage-
            interleaved pairs so every engine always has two independent ops
            queued. Yields between stage waves."""
            s2s, sp2s = state.pop(g)
            p7 = tpool.tile([NE, GROUP * CHUNK], DT, tag="p7")
            for pair in ((0, 1), (2, 3), (4, 5), (6,)):
                x0s, g0s, x1s, g1s, x2s, g2s = {}, {}, {}, {}, {}, {}
                for e in pair:
                    x0 = x0s[e] = ppool.tile([128, GROUP * CHUNK], DT, tag="exp", name=f"x0_{g}_{e}")
                    for ci in range(GROUP):
                        sl = x0[:, ci * CHUNK:(ci + 1) * CHUNK]
                        nc.tensor.matmul(sl, w_e0a[e], s2s[ci][:], start=True, stop=False)
                        nc.tensor.matmul(sl, w_e0b[e], sp2s[ci][:], start=False, stop=True)
                yield
                for e in pair:
                    g0 = g0s[e] = gpool.tile([EXP, GROUP * CHUNK], F32R, tag="g0", name=f"g0_{g}_{e}")
                    wrap_sin(g0[:], x0s[e][:], EXP, GROUP * CHUNK)
                yield
                for e in pair:
                    x1 = x1s[e] = ppool.tile([128, GROUP * CHUNK], DT, tag="exp", name=f"x1_{g}_{e}")
                    for ci in range(GROUP):
                        nc.tensor.matmul(
                            x1[:, ci * CHUNK:(ci + 1) * CHUNK], w_e1[e],
                            g0s[e][:, ci * CHUNK:(ci + 1) * CHUNK], start=True, stop=True,
                        )
                yield
                for e in pair:
                    g1 = g1s[e] = gpool.tile([EXP, GROUP * CHUNK], F32R, tag="g1", name=f"g1_{g}_{e}")
                    wrap_sin(g1[:], x1s[e][:], EXP, GROUP * CHUNK)
                yield
                for e in pair:
                    x2 = x2s[e] = ppool.tile([128, GROUP * CHUNK], DT, tag="exp", name=f"x2_{g}_{e}")
                    for ci in range(GROUP):
                        nc.tensor.matmul(
                            x2[:, ci * CHUNK:(ci + 1) * CHUNK], w_e2[e],
                            g1s[e][:, ci * CHUNK:(ci + 1) * CHUNK], start=True, stop=True,
                        )
                yield
                for e in pair:
                    g2 = g2s[e] = gpool.tile([EXP, GROUP * CHUNK], F32R, tag="g2", name=f"g2_{g}_{e}")
                    wrap_sin(g2[:], x2s[e][:], EXP, GROUP * CHUNK)
                yield
                for e in pair:
                    for ci in range(GROUP):
                        nc.tensor.matmul(
                            p7[0:NE, ci * CHUNK:(ci + 1) * CHUNK], w_eo[e],
                            g2s[e][:, ci * CHUNK:(ci + 1) * CHUNK],
                            start=(e == 0), stop=(e == NE - 1),
                        )
                yield

            # preds -> point-major PT via PE transpose
            for ci in range(GROUP):
                c = g * GROUP + ci
                tmp7 = spool.tile([NE, CHUNK], DT, tag="tmp7")
                nc.scalar.activation(tmp7[:], p7[0:NE, ci * CHUNK:(ci + 1) * CHUNK], AF.Copy)
                tp = ppool.tile([128, NSUB * NE], DT, tag="trunk")
                for s in range(NSUB):
                    nc.tensor.transpose(
                        tp[:, s * NE:(s + 1) * NE],
                        tmp7[:, s * 128:(s + 1) * 128],
                        w_id[0:NE, 0:NE],
                    )
                nc.scalar.activation(PT[:, c * NSUB * NE:(c + 1) * NSUB * NE], tp[:, 0:NSUB * NE], AF.Copy)
                yield

        _STOP = object()

        # ===== routing + combine, column-sliced so the first half can be
        # emitted mid-pipeline (hidden under groups ngroup/2.. expert work).
        def routing_block(jl, jh, sfx):
            w = jh - jl
            LTs = LT[:, jl * NE:jh * NE]
            PTs = PT[:, jl * NE:jh * NE]
            LT3 = LTs.rearrange("p (j e) -> p j e", e=NE)

            def etree(op, src3, width_tag):
                m4 = rpool.tile([128, w * 4], DT, tag=f"{width_tag}4{sfx}")
                m43 = m4[:].rearrange("p (j e) -> p j e", e=4)
                nc.vector.tensor_tensor(m43, src3[:, :, 0:4], src3[:, :, 3:7], op)
                m2 = rpool.tile([128, w * 2], DT, tag=f"{width_tag}2{sfx}")
                m23 = m2[:].rearrange("p (j e) -> p j e", e=2)
                nc.vector.tensor_tensor(m23, m43[:, :, 0:2], m43[:, :, 2:4], op)
                m1 = rpool.tile([128, w], DT, tag=f"{width_tag}1{sfx}")
                m13 = m1[:].rearrange("p (j e) -> p j e", e=1)
                nc.vector.tensor_tensor(m13, m23[:, :, 0:1], m23[:, :, 1:2], op)
                return m1

            def erep(m1, tag):
                r_ = rpool.tile([128, w * NE], DT, tag=f"{tag}{sfx}")
                r3 = r_[:].rearrange("p (j e) -> p j e", e=NE)
                m13 = m1[:].rearrange("p (j e) -> p j e", e=1)
                nc.gpsimd.tensor_copy(r3[:, :, 0:1], m13)
                nc.gpsimd.tensor_copy(r3[:, :, 1:2], r3[:, :, 0:1])
                nc.gpsimd.tensor_copy(r3[:, :, 2:4], r3[:, :, 0:2])
                nc.gpsimd.tensor_copy(r3[:, :, 4:7], r3[:, :, 1:4])
                return r_, r3

            mx1 = etree(ALU.max, LT3, "mxa")
            rep1, _ = erep(mx1, "rep1")
            ge1 = rpool.tile([128, w * NE], DT, tag=f"ge1{sfx}")
            nc.vector.tensor_tensor(ge1[:], LTs, rep1[:], ALU.is_ge)
            maskd = rpool.tile([128, w * NE], DT, tag=f"maskd{sfx}")
            nc.vector.scalar_tensor_tensor(
                maskd[:], ge1[:], BIG, LTs, op0=ALU.mult, op1=ALU.subtract
            )
            mn = etree(ALU.min, maskd[:].rearrange("p (j e) -> p j e", e=NE), "mna")
            mx2 = rpool.tile([128, w], DT, tag=f"mx2{sfx}")
            nc.vector.tensor_scalar_mul(mx2[:], mn[:], -1.0)
            rep2, _ = erep(mx2, "rep2")
            keep = rpool.tile([128, w * NE], DT, tag=f"keep{sfx}")
            nc.vector.tensor_tensor(keep[:], LTs, rep2[:], ALU.is_ge)

            ex = rpool.tile([128, w * NE], DT, tag=f"ex{sfx}")
            nc.scalar.activation(ex[:], LTs, AF.Exp)
            ew = rpool.tile([128, w * NE], DT, tag=f"ew{sfx}")
            nc.vector.tensor_tensor(ew[:], ex[:], keep[:], ALU.mult)
            wp = rpool.tile([128, w * NE], DT, tag=f"wp{sfx}")
            nc.vector.tensor_tensor(wp[:], ew[:], PTs, ALU.mult)

            den = rpool.tile([128, w], DT, tag=f"den{sfx}")
            nc.vector.tensor_reduce(
                den[:], ew[:].rearrange("p (j e) -> p j e", e=NE),
                mybir.AxisListType.X, ALU.add,
            )
            num = rpool.tile([128, w], DT, tag=f"num{sfx}")
            nc.vector.tensor_reduce(
                num[:], wp[:].rearrange("p (j e) -> p j e", e=NE),
                mybir.AxisListType.X, ALU.add,
            )
            rec = rpool.tile([128, w], DT, tag=f"rec{sfx}")
            scratch = rpool.tile([128, w], DT, tag=f"recs{sfx}")
            nc.vector.reciprocal_approx_accurate(rec[:], den[:], scratch[:])
            outv = rpool.tile([128, w], DT, tag=f"outv{sfx}")
            nc.vector.tensor_tensor(outv[:], num[:], rec[:], ALU.mult)

            # transpose [128 q, w] -> [w, 128 q] and store
            tp = ppool.tile([128, 128], DT, tag="exp")
            nc.tensor.transpose(tp[0:w, 0:128], outv[:, 0:w], w_id[:])
            osb = rpool.tile([w, 128], DT, tag=f"osb{sfx}")
            nc.scalar.activation(osb[:], tp[0:w, 0:128], AF.Copy)
            nc.sync.dma_start(
                out_d.rearrange("(j q) -> j q", q=128)[jl:jh, :], osb[:],
            )

        # software pipeline: experts(g) interleaved with trunk(g+1); the
        # per-engine queues execute in program order, so the emission order
        # must alternate independent work to keep every engine fed.
        # expert stream: ~30 yields/group; trunk: ~14 -> interleave 2:1.
        tg = trunk_gen(0)
        while next(tg, _STOP) is not _STOP:
            pass
        for g in range(ngroup):
            eg = experts_gen(g)
            tg = trunk_gen(g + 1) if g + 1 < ngroup else None
            alive = True
            while alive:
                e_live = next(eg, _STOP) is not _STOP
                e_live = (next(eg, _STOP) is not _STOP) or e_live
                t_live = tg is not None and next(tg, _STOP) is not _STOP
                alive = e_live or t_live
            if g == ngroup // 2 - 1:
                # columns 0..nj/2 are final; overlap their routing with the
                # remaining groups' expert compute.
                routing_block(0, nj // 2, "a")
        routing_block(nj // 2, nj, "b")

    nc.compile()
    return nc


_W_NAMES = [
    "enc_W1", "enc_W2", "pol_W0", "pol_W1", "pol_W2", "pol_Wl",
    "exp_W0", "exp_W1", "exp_W2", "exp_Wo",
]


def _prep_weights(inputs):
    """Per-core weight tensors (identical on every core)."""
    f32 = np.float32
    S30 = f32(OMEGA)
    SP = f32(OMEGA / (2 * np.pi))

    # pe freq matrix in period units: col j=i*6+k (sin), 24+j (cos) = 2^(k-1)
    pe_bs = np.zeros((IN_F, 48), f32)
    for i in range(IN_F):
        for k in range(NUM_FREQ):
            pe_bs[i, i * NUM_FREQ + k] = 2.0 ** (k - 1)
            pe_bs[i, 24 + i * NUM_FREQ + k] = 2.0 ** (k - 1)
    pe_shift = np.zeros((48, 1), f32)
    pe_shift[24:48] = 0.25
    pe_bias = (pe_shift * f32(2 * np.pi)).astype(f32)

    # enc_W1 rows permuted to [sin/cos(48); x(4)], scaled to period units
    encW1 = np.asarray(inputs["enc_W1"], f32)
    encW1p = np.concatenate([encW1[4:52], encW1[0:4]], axis=0) * SP

    d = {
        "pe_bs": pe_bs,
        "pe_shift": pe_shift,
        "pe_bias": pe_bias,
        "encW1a": np.ascontiguousarray(encW1p[0:48]).astype(f32),
        "encW1b": np.ascontiguousarray(encW1p[48:52]).astype(f32),
        "encW2r": (np.asarray(inputs["enc_W2"], f32) * S30),
        "polW0p": (np.asarray(inputs["pol_W0"], f32)[0:IN_F] * SP),
        "polW1r": (np.asarray(inputs["pol_W1"], f32) * S30),
        "polW2r": (np.asarray(inputs["pol_W2"], f32) * S30),
        "polWl": np.asarray(inputs["pol_Wl"], f32),
        "eW0a": np.ascontiguousarray(
            (np.asarray(inputs["exp_W0"], f32)[:, 0:ENC, :] * S30)
            .transpose(1, 0, 2).reshape(ENC, NE * EXP)),
        "eW0b": np.ascontiguousarray(
            (np.asarray(inputs["exp_W0"], f32)[:, ENC:ENC + POL, :] * S30)
            .transpose(1, 0, 2).reshape(POL, NE * EXP)),
        "eW1": np.ascontiguousarray(
            (np.asarray(inputs["exp_W1"], f32) * S30)
            .transpose(1, 0, 2).reshape(EXP, NE * EXP)),
        "eW2": np.ascontiguousarray(
            (np.asarray(inputs["exp_W2"], f32) * S30)
            .transpose(1, 0, 2).reshape(EXP, NE * EXP)),
        "ident": np.eye(128, dtype=f32),
    }
    eWo = np.zeros((EXP, NE, NE), f32)
    for e in range(NE):
        eWo[:, e, e] = np.asarray(inputs["exp_Wo"], f32)[e, :, 0]
    d["eWo"] = eWo.reshape(EXP, NE * NE)

    # biases are structurally zero in this model; the kernel folds none.
    for b in ["enc_b1", "enc_b2", "pol_b0", "pol_b1", "pol_b2", "pol_bl",
              "exp_b0", "exp_b1", "exp_b2", "exp_bo"]:
        assert not np.any(np.asarray(inputs[b])), f"nonzero bias {b} unsupported"

    return d


def _runtime():
    """Build (once) the bass module and a persistent jitted 8-core executable."""
    if "sharded" in _RT:
        return _RT

    import jax
    import concourse.mybir as mybir
    from concourse.bass2jax import _bass_exec_p, install_neuronx_cc_hook

    import warnings

    with warnings.catch_warnings():
        warnings.simplefilter("ignore")
        from jax.experimental.shard_map import shard_map
    from jax.sharding import Mesh, NamedSharding, PartitionSpec

    install_neuronx_cc_hook()

    nc = _build(NPC)

    partition_name = nc.partition_id_tensor.name if nc.partition_id_tensor else None
    in_names, out_names, out_avals = [], [], []
    for alloc in nc.m.functions[0].allocations:
        if not isinstance(alloc, mybir.MemoryLocationSet):
            continue
        name = alloc.memorylocations[0].name
        if alloc.kind == "ExternalInput":
            if name != partition_name:
                in_names.append(name)
        elif alloc.kind == "ExternalOutput":
            out_names.append(name)
            out_avals.append(
                jax.core.ShapedArray(tuple(alloc.tensor_shape), mybir.dt.np(alloc.dtype))
            )
    n_params = len(in_names)
    n_outs = len(out_names)
    in_names_all = list(in_names) + list(out_names)
    if partition_name is not None:
        in_names_all.append(partition_name)
    donate = tuple(range(n_params, n_params + n_outs))

    def _body(*args):
        operands = list(args)
        if partition_name is not None:
            from concourse.bass2jax import partition_id_tensor

            operands.append(partition_id_tensor())
        outs = _bass_exec_p.bind(
            *operands,
            out_avals=tuple(out_avals),
            in_names=tuple(in_names_all),
            out_names=tuple(out_names),
            lowering_input_output_aliases=(),
            sim_require_finite=True,
            sim_require_nnan=True,
            nc=nc,
        )
        return tuple(outs)

    devices = jax.devices()[:N_CORES]
    mesh = Mesh(np.asarray(devices), ("core",))
    spec = PartitionSpec("core")
    sharded = jax.jit(
        shard_map(
            _body,
            mesh=mesh,
            in_specs=(spec,) * (n_params + n_outs),
            out_specs=(spec,) * n_outs,
            check_rep=False,
        ),
        donate_argnums=donate,
        keep_unused=True,
    )

    _RT.update(
        nc=nc, jax=jax, mesh=mesh, shard=NamedSharding(mesh, spec),
        sharded=sharded, in_names=in_names, out_avals=out_avals,
    )
    return _RT


def _fingerprint(inputs):
    acc = 0
    for k in _W_NAMES:
        a = np.ascontiguousarray(np.asarray(inputs[k], np.float32))
        acc = zlib.crc32(a.view(np.uint8).reshape(-1), acc)
    return acc


def kernel(**inputs):
    rt = _runtime()
    jax = rt["jax"]

    # enqueue the x upload first so the transfer overlaps the fingerprint
    x = np.ascontiguousarray(np.asarray(inputs["x"], np.float32))
    assert x.shape == (N_TOTAL, IN_F)
    x_dev = jax.device_put(x, rt["shard"])

    wfp = _fingerprint(inputs)
    if rt.get("wfp") != wfp:
        wd = _prep_weights(inputs)
        devw = {}
        for name in rt["in_names"]:
            if name == "x":
                continue
            a = wd[name]
            glob = np.concatenate([a] * N_CORES, axis=0)
            devw[name] = jax.device_put(glob, rt["shard"])
        rt["devw"] = devw
        rt["wfp"] = wfp
        rt["prev_out"] = None

    donate_buf = rt.get("prev_out")
    if donate_buf is None:
        donate_buf = jax.device_put(np.zeros(N_TOTAL, np.float32), rt["shard"])

    args = [x_dev if n == "x" else rt["devw"][n] for n in rt["in_names"]]
    rt["prev_out"] = None  # donate_buf is consumed even if the call fails
    outs = rt["sharded"](*args, donate_buf)
    rt["prev_out"] = outs[0]
    res = np.asarray(outs[0])
    return res.reshape(N_TOTAL, 1).astype(np.float32, copy=False)
